# revision 1
# baseline (speedup 1.0000x reference)
"""GRU observation-cell kernel for Trainium2 (8 NeuronCores).

Reference computation:
    x = X_obs.reshape(M, 128); hs = h[i_obs]
    h_new = GRUCell(x, hs)  (torch gate order r,z,n)
    out = h.at[i_obs].set(h_new)

Device strategy (data parallel over observed rows, per sharding hint):
  - M=20000 observed rows sharded 2500/core across 8 cores.
  - Gates-on-partitions layout: host supplies x^T [128,2500] and hs^T
    [256,2500] per core, so every matmul operand is already contract-
    dim-major and no on-chip transposes are needed.
  - Matmuls run as float32r (fp32 bits, full PE rate at >=256 moving dim),
    accumulating fp32 in PSUM. For the r/z gates both x@W_ih^T and
    hs@W_hh^T accumulate into the same PSUM bank, so the gi+gh add is free.
  - n gate: t1 = r*(h_n + b_hh_n) is one fused scalar_tensor_tensor on
    DVE; the + i_n add rides on the PE as an identity-matmul accumulate
    into the i_n PSUM bank, and tanh(+b_ih_n bias) drains that bank.
  - Emission order r-gates -> n-gates -> z-gates shortens the critical
    path into the DVE chain.
  - The gather h[i_obs] / scatter out[i_obs] is part of host-side
    shard/unshard (i_obs indexes rows; untouched rows pass through).
"""

import numpy as np

N, H, IN2, M, NCORES = 100000, 256, 128, 20000, 8
MC = M // NCORES        # 2500 observed rows per core
RT = 500                # rows per tile (<=512 fp32 PSUM bank limit)
NRT = MC // RT          # 5 row tiles per core
G3 = 3 * H              # 768 stacked gates (r,z,n)

_compiled = {}


def _build_nc():
    from contextlib import ExitStack

    from concourse import bacc
    import concourse.mybir as mybir
    from concourse.tile import TileContext

    dt = mybir.dt
    f32 = dt.float32
    f32r = dt.float32r
    AF = mybir.ActivationFunctionType
    ALU = mybir.AluOpType

    nc = bacc.Bacc(None, target_bir_lowering=False)

    xT_d = nc.dram_tensor("xT", [IN2, MC], f32r, kind="ExternalInput")
    hT_d = nc.dram_tensor("hT", [H, MC], f32r, kind="ExternalInput")
    wiT_d = nc.dram_tensor("wiT", [IN2, G3], f32r, kind="ExternalInput")
    whT_d = nc.dram_tensor("whT", [H, G3], f32r, kind="ExternalInput")
    id_d = nc.dram_tensor("ident", [128, 128], f32r, kind="ExternalInput")
    brz_d = nc.dram_tensor("brz", [2 * H], f32, kind="ExternalInput")
    bin_d = nc.dram_tensor("bin", [H], f32, kind="ExternalInput")
    bhn_d = nc.dram_tensor("bhn", [H], f32, kind="ExternalInput")
    out_d = nc.dram_tensor("houtT", [H, MC], f32, kind="ExternalOutput")

    with TileContext(nc) as tc, ExitStack() as ctx:
        const = ctx.enter_context(tc.tile_pool(name="const", bufs=1))
        xin = ctx.enter_context(tc.tile_pool(name="xin", bufs=3))
        hin = ctx.enter_context(tc.tile_pool(name="hin", bufs=3))
        psum = ctx.enter_context(tc.tile_pool(name="psum", bufs=2, space="PSUM"))
        work = ctx.enter_context(tc.tile_pool(name="work", bufs=3))
        outp = ctx.enter_context(tc.tile_pool(name="outp", bufs=4))

        # --- constants / weights (loaded once) ---
        brz_sb = const.tile([128, 4], f32, tag="brz")
        nc.scalar.dma_start(
            out=brz_sb[:], in_=brz_d[:].rearrange("(g p) -> p g", p=128)
        )
        bin_sb = const.tile([128, 2], f32, tag="bin")
        nc.scalar.dma_start(
            out=bin_sb[:], in_=bin_d[:].rearrange("(g p) -> p g", p=128)
        )
        bhn_sb = const.tile([128, 2], f32, tag="bhn")
        nc.scalar.dma_start(
            out=bhn_sb[:], in_=bhn_d[:].rearrange("(g p) -> p g", p=128)
        )
        wi_sb = const.tile([IN2, G3], f32r, tag="wi")
        nc.sync.dma_start(out=wi_sb[:], in_=wiT_d[:, :])
        # first row-tile data right behind wi so the PE can start early
        x0 = xin.tile([IN2, RT], f32r, tag="x", name="x0")
        nc.sync.dma_start(out=x0[:], in_=xT_d[:, 0:RT])
        h00 = hin.tile([128, RT], f32r, tag="h0", name="h00")
        nc.sync.dma_start(out=h00[:], in_=hT_d[0:128, 0:RT])
        h01 = hin.tile([128, RT], f32r, tag="h1", name="h01")
        nc.sync.dma_start(out=h01[:], in_=hT_d[128:256, 0:RT])
        wh0_sb = const.tile([128, G3], f32r, tag="wh0")
        nc.sync.dma_start(out=wh0_sb[:], in_=whT_d[0:128, :])
        wh1_sb = const.tile([128, G3], f32r, tag="wh1")
        nc.sync.dma_start(out=wh1_sb[:], in_=whT_d[128:256, :])
        id_sb = const.tile([128, 128], f32r, tag="ident")
        nc.sync.dma_start(out=id_sb[:], in_=id_d[:, :])


        for t in range(NRT):
            c0 = t * RT
            if t == 0:
                x_t, h_t = x0, [h00, h01]
            else:
                x_t = xin.tile([IN2, RT], f32r, tag="x")
                nc.sync.dma_start(out=x_t[:], in_=xT_d[:, c0 : c0 + RT])
                h_t = [None, None]
                for j in range(2):
                    h_t[j] = hin.tile([128, RT], f32r, tag=f"h{j}", name=f"h_t{j}")
                    nc.sync.dma_start(
                        out=h_t[j][:], in_=hT_d[j * 128 : (j + 1) * 128, c0 : c0 + RT]
                    )

            def gate_mm(ps, gs):
                nc.tensor.matmul(
                    ps[:], lhsT=wi_sb[:, gs], rhs=x_t[:], start=True, stop=False
                )
                nc.tensor.matmul(
                    ps[:], lhsT=wh0_sb[:, gs], rhs=h_t[0][:], start=False, stop=False
                )
                nc.tensor.matmul(
                    ps[:], lhsT=wh1_sb[:, gs], rhs=h_t[1][:], start=False, stop=True
                )

            def sig(g):
                ps = psum.tile([128, RT], f32, tag="ps", bufs=4, name="ps_rz")
                gate_mm(ps, slice(g * 128, (g + 1) * 128))
                sg = work.tile([128, RT], f32, tag=f"sig{g}", name="sg")
                nc.scalar.activation(
                    out=sg[:], in_=ps[:], func=AF.Sigmoid, bias=brz_sb[:, g : g + 1]
                )
                return sg

            # r gates first: they head the DVE chain.
            r_sig = [sig(0), sig(1)]
            if t == NRT - 1:
                z_sig = [sig(2), sig(3)]
            n_t = [None, None]
            for j in range(2):
                gs = slice(2 * H + j * 128, 2 * H + (j + 1) * 128)
                ps_in = psum.tile([128, RT], f32, tag="psin", bufs=2, name="ps_in")
                nc.tensor.matmul(
                    ps_in[:], lhsT=wi_sb[:, gs], rhs=x_t[:], start=True, stop=False
                )
                ps_hn = psum.tile([128, RT], f32, tag="pshn", bufs=2, name="ps_hn")
                nc.tensor.matmul(
                    ps_hn[:], lhsT=wh0_sb[:, gs], rhs=h_t[0][:], start=True, stop=False
                )
                nc.tensor.matmul(
                    ps_hn[:], lhsT=wh1_sb[:, gs], rhs=h_t[1][:], start=False, stop=True
                )

                t1 = work.tile([128, RT], f32r, tag=f"t1_{j}", name="t1")
                nc.vector.scalar_tensor_tensor(
                    out=t1[:],
                    in0=ps_hn[:],
                    scalar=bhn_sb[:, j : j + 1],
                    in1=r_sig[j][:],
                    op0=ALU.add,
                    op1=ALU.mult,
                )
                nc.tensor.matmul(
                    ps_in[:], lhsT=id_sb[:], rhs=t1[:], start=False, stop=True
                )
                n_t[j] = work.tile([128, RT], f32, tag=f"n_{j}", name="n_t")
                nc.scalar.activation(
                    out=n_t[j][:], in_=ps_in[:], func=AF.Tanh,
                    bias=bin_sb[:, j : j + 1],
                )

            # z gates late: only needed by the final blend.
            if t != NRT - 1:
                z_sig = [sig(2), sig(3)]

            for j in range(2):
                d_t = work.tile([128, RT], f32, tag=f"d_{j}", name="d_t")
                nc.vector.tensor_sub(
                    out=d_t[:], in0=h_t[j][:].bitcast(f32), in1=n_t[j][:]
                )
                e_t = work.tile([128, RT], f32, tag=f"e_{j}", name="e_t")
                nc.vector.tensor_mul(out=e_t[:], in0=z_sig[j][:], in1=d_t[:])
                ho = outp.tile([128, RT], f32, tag=f"ho_{j}", name="ho")
                nc.vector.tensor_add(out=ho[:], in0=n_t[j][:], in1=e_t[:])
                nc.sync.dma_start(
                    out=out_d[j * 128 : (j + 1) * 128, c0 : c0 + RT], in_=ho[:]
                )

    nc.compile()
    return nc


def _get_nc():
    if "nc" not in _compiled:
        _compiled["nc"] = _build_nc()
    return _compiled["nc"]


def _make_in_maps(h, X_obs, i_obs, W_ih, W_hh, b_ih, b_hh):
    f = np.float32
    x = np.asarray(X_obs, f).reshape(M, IN2)
    hs = np.asarray(h, f)[np.asarray(i_obs)]
    xT = np.ascontiguousarray(x.T)
    hT = np.ascontiguousarray(hs.T)
    wiT = np.ascontiguousarray(np.asarray(W_ih, f).T)
    whT = np.ascontiguousarray(np.asarray(W_hh, f).T)
    ident = np.eye(128, dtype=f)
    b_ih = np.asarray(b_ih, f)
    b_hh = np.asarray(b_hh, f)
    brz = (b_ih[: 2 * H] + b_hh[: 2 * H]).astype(f)
    bin_ = np.ascontiguousarray(b_ih[2 * H :])
    bhn = np.ascontiguousarray(b_hh[2 * H :])
    in_maps = []
    for c in range(NCORES):
        cols = slice(c * MC, (c + 1) * MC)
        in_maps.append(
            {
                "xT": np.ascontiguousarray(xT[:, cols]),
                "hT": np.ascontiguousarray(hT[:, cols]),
                "wiT": wiT,
                "whT": whT,
                "ident": ident,
                "brz": brz,
                "bin": bin_,
                "bhn": bhn,
            }
        )
    return in_maps


def run_on_device(h, X_obs, i_obs, W_ih, W_hh, b_ih, b_hh, **run_kwargs):
    """Returns (h_new [M,H] fp32, BassKernelResults)."""
    from concourse.bass_utils import run_bass_kernel_spmd

    in_maps = _make_in_maps(h, X_obs, i_obs, W_ih, W_hh, b_ih, b_hh)
    res = run_bass_kernel_spmd(_get_nc(), in_maps, list(range(NCORES)), **run_kwargs)
    h_new = np.concatenate([r["houtT"].T for r in res.results], axis=0)
    return h_new, res


def kernel(h, X_obs, i_obs, W_ih, W_hh, b_ih, b_hh):
    h = np.asarray(h, np.float32)
    i_obs = np.asarray(i_obs)
    h_new, _ = run_on_device(h, X_obs, i_obs, W_ih, W_hh, b_ih, b_hh)
    out = h.copy()
    out[i_obs] = h_new
    return out



# revision 5
# speedup vs baseline: 1.1913x; 1.1913x over previous
"""GRU observation-cell kernel for Trainium2 (8 NeuronCores).

Reference computation:
    x = X_obs.reshape(M, 128); hs = h[i_obs]
    h_new = GRUCell(x, hs)  (torch gate order r,z,n)
    out = h.at[i_obs].set(h_new)

Device strategy (data parallel over observed rows, per sharding hint):
  - M=20000 observed rows sharded 2500/core across 8 cores, tiled in 5
    column-tiles of 500 rows (gates-on-partitions layout).
  - r/z gates: fp8e4 DoubleRow matmuls (2x PE rate). Moving operand is
    u = [x^T; const; hs^T(lo); hs^T(hi)] (4 contract groups of 128); the
    const group carries a single 1.0 row so the r/z (and i_n) biases ride
    in the matmul as an extra contract row -> PSUM already contains
    16*(gates+bias), letting one activation op drain two PSUM banks
    (the per-partition activation bias could not express per-bank biases).
  - n gate: i_n via one fp8 DoubleRow matmul (bias folded the same way);
    h_n in fp16 (tanh has slope 1, keep it accurate); t1 = r*(h_n+b_hn)
    as scalar_tensor_tensor (j=0 on DVE, j=1 on GPSIMD); the i_n + t1 add
    rides the PE as an fp16 identity-matmul accumulate; tanh drains it.
  - All fp8/fp16 weights are pre-scaled by 16 on host (fp8e4 subnormal
    avoidance); activations apply scale=1/16.
  - Blend (1-z)n + z*hs: three fp16 tensor-tensor ops on DVE (2x mode).
  - Input DMAs issued from SP (HWDGE), output DMAs from GPSIMD (SWDGE)
    to spread descriptor-generation across both paths.
"""

import numpy as np

N, H, IN2, M, NCORES = 100000, 256, 128, 20000, 8
MC = M // NCORES        # 2500 observed rows per core
RT = 500                # rows per column-tile (<=512 fp32 PSUM bank limit)
NRT = MC // RT          # 5 row tiles per core
S = 16.0                # fp8 weight pre-scale; activations apply 1/S

_compiled = {}


def _build_nc():
    from contextlib import ExitStack

    from concourse import bacc
    import concourse.mybir as mybir
    from concourse.tile import TileContext

    dt = mybir.dt
    f32 = dt.float32
    f16 = dt.float16
    f8 = dt.float8e4
    AF = mybir.ActivationFunctionType
    ALU = mybir.AluOpType
    DR = mybir.MatmulPerfMode.DoubleRow

    nc = bacc.Bacc(None, target_bir_lowering=False)

    u_d = nc.dram_tensor("u", [NRT, 128, 4, RT], f8, kind="ExternalInput")
    hb_d = nc.dram_tensor("hb", [NRT, 128, 2, RT], f16, kind="ExternalInput")
    wrz_d = nc.dram_tensor("wrz", [128, 16, 128], f8, kind="ExternalInput")
    win_d = nc.dram_tensor("win", [128, 4, 128], f8, kind="ExternalInput")
    whn_d = nc.dram_tensor("whn", [128, 4, 128], f16, kind="ExternalInput")
    id_d = nc.dram_tensor("ident", [128, 128], f16, kind="ExternalInput")
    bhn_d = nc.dram_tensor("bhn", [128, 2], f32, kind="ExternalInput")
    out_d = nc.dram_tensor("hout", [NRT, 128, 2, RT], f16, kind="ExternalOutput")

    with TileContext(nc) as tc, ExitStack() as ctx:
        const = ctx.enter_context(tc.tile_pool(name="const", bufs=1))
        uin = ctx.enter_context(tc.tile_pool(name="uin", bufs=2))
        hin = ctx.enter_context(tc.tile_pool(name="hin", bufs=2))
        rzp = ctx.enter_context(tc.tile_pool(name="rzp", bufs=2))
        wrk = ctx.enter_context(tc.tile_pool(name="wrk", bufs=2))
        outp = ctx.enter_context(tc.tile_pool(name="outp", bufs=NRT))
        psum = ctx.enter_context(tc.tile_pool(name="psum", bufs=1, space="PSUM"))

        # --- constants / weights, first tile inputs ---
        wrz_sb = const.tile([128, 16, 128], f8, tag="wrz")
        nc.sync.dma_start(out=wrz_sb[:], in_=wrz_d[:, :, :])
        u_t = [None] * NRT
        hb_t = [None] * NRT
        u_t[0] = uin.tile([128, 4, RT], f8, tag="u", name="u0")
        nc.sync.dma_start(out=u_t[0][:], in_=u_d[0])
        hb_t[0] = hin.tile([128, 2, RT], f16, tag="hb", name="hb0")
        nc.sync.dma_start(out=hb_t[0][:], in_=hb_d[0])
        win_sb = const.tile([128, 4, 128], f8, tag="win")
        nc.sync.dma_start(out=win_sb[:], in_=win_d[:, :, :])
        whn_sb = const.tile([128, 4, 128], f16, tag="whn")
        nc.sync.dma_start(out=whn_sb[:], in_=whn_d[:, :, :])
        id_sb = const.tile([128, 128], f16, tag="ident")
        nc.sync.dma_start(out=id_sb[:], in_=id_d[:, :])
        bhn_sb = const.tile([128, 2], f32, tag="bhn")
        nc.sync.dma_start(out=bhn_sb[:], in_=bhn_d[:, :])

        r_t = [None] * NRT
        z_t = [None] * NRT
        n_t = [None] * NRT
        t1_t = [[None, None] for _ in range(NRT)]
        inps_t = [None] * NRT
        ho_t = [None] * NRT

        def rz_mms(t, gh, ps):
            # gh in 0..3 -> r0,r1,z0,z1; two DoubleRow matmuls: (x,const)+(h0,h1)
            nc.tensor.matmul(
                ps[:], lhsT=wrz_sb[:, 4 * gh : 4 * gh + 2, :],
                rhs=u_t[t][:, 0:2, :], start=True, stop=False, perf_mode=DR,
            )
            nc.tensor.matmul(
                ps[:], lhsT=wrz_sb[:, 4 * gh + 2 : 4 * gh + 4, :],
                rhs=u_t[t][:, 2:4, :], start=False, stop=True, perf_mode=DR,
            )

        def id_mms(t):
            # i_n PSUM accumulation finishes with the t1 identity-matmul ride
            for j in range(2):
                nc.tensor.matmul(
                    inps_t[t][:, j, 0:RT], lhsT=id_sb[:], rhs=t1_t[t][j][:],
                    start=False, stop=True,
                )

        for t in range(NRT):
            if t + 1 < NRT:
                u_t[t + 1] = uin.tile([128, 4, RT], f8, tag="u", name=f"u{t+1}")
                nc.sync.dma_start(out=u_t[t + 1][:], in_=u_d[t + 1])
                hb_t[t + 1] = hin.tile([128, 2, RT], f16, tag="hb", name=f"hb{t+1}")
                nc.sync.dma_start(out=hb_t[t + 1][:], in_=hb_d[t + 1])

            rps = psum.tile([128, 2, 512], f32, tag="rps", name="rps")
            zps = psum.tile([128, 2, 512], f32, tag="zps", name="zps")
            for j in range(2):
                rz_mms(t, j, rps[:, j, 0:RT])
            for j in range(2):
                rz_mms(t, 2 + j, zps[:, j, 0:RT])
            if t > 0:
                id_mms(t - 1)
            inps_t[t] = psum.tile([128, 2, 512], f32, tag="inps", name="inps")
            hnps = psum.tile([128, 2, 512], f32, tag="hnps", name="hnps")
            for j in range(2):
                nc.tensor.matmul(
                    inps_t[t][:, j, 0:RT], lhsT=win_sb[:, 2 * j : 2 * j + 2, :],
                    rhs=u_t[t][:, 0:2, :], start=True, stop=False, perf_mode=DR,
                )
            for j in range(2):
                for k in range(2):
                    nc.tensor.matmul(
                        hnps[:, j, 0:RT], lhsT=whn_sb[:, 2 * j + k, :],
                        rhs=hb_t[t][:, k, :], start=(k == 0), stop=(k == 1),
                    )

            # --- activations: R_t, Z_t, then B_{t-1} ---
            r_t[t] = rzp.tile([128, 2, RT], f16, tag="r", name="r_t")
            nc.scalar.activation(
                out=r_t[t][:], in_=rps[:, :, 0:RT], func=AF.Sigmoid, scale=1.0 / S
            )
            z_t[t] = rzp.tile([128, 2, RT], f16, tag="z", name="z_t")
            nc.scalar.activation(
                out=z_t[t][:], in_=zps[:, :, 0:RT], func=AF.Sigmoid, scale=1.0 / S
            )
            if t > 0:
                n_t[t - 1] = wrk.tile([128, 2, RT], f16, tag="n", name="n_t")
                nc.scalar.activation(
                    out=n_t[t - 1][:], in_=inps_t[t - 1][:, :, 0:RT],
                    func=AF.Tanh, scale=1.0 / S,
                )

            # --- t1_j = (h_n + 16*b_hn) * r on DVE (GPSIMD cannot read PSUM) ---
            for j in range(2):
                t1_t[t][j] = wrk.tile([128, RT], f16, tag=f"t1_{j}", name="t1")
                nc.vector.scalar_tensor_tensor(
                    out=t1_t[t][j][:], in0=hnps[:, j, 0:RT],
                    scalar=bhn_sb[:, j : j + 1], in1=r_t[t][:, j, :],
                    op0=ALU.add, op1=ALU.mult,
                )

            # --- blend of previous tile: ho = n + z*(hb - n) ---
            if t > 0:
                tp = t - 1
                d_t = wrk.tile([128, 2, RT], f16, tag="d", name="d_t")
                nc.vector.tensor_sub(out=d_t[:], in0=hb_t[tp][:], in1=n_t[tp][:])
                e_t = wrk.tile([128, 2, RT], f16, tag="e", name="e_t")
                nc.vector.tensor_mul(out=e_t[:], in0=z_t[tp][:], in1=d_t[:])
                ho_t[tp] = outp.tile([128, 2, RT], f16, tag="ho", name="ho")
                # final add on the otherwise-idle GPSIMD engine (SBUF only)
                nc.gpsimd.tensor_add(out=ho_t[tp][:], in0=n_t[tp][:], in1=e_t[:])

        # --- tail: last tile's n/blend, then all output DMAs from SP ---
        t = NRT - 1
        id_mms(t)
        n_t[t] = wrk.tile([128, 2, RT], f16, tag="n", name="n_last")
        nc.scalar.activation(
            out=n_t[t][:], in_=inps_t[t][:, :, 0:RT], func=AF.Tanh, scale=1.0 / S
        )
        d_t = wrk.tile([128, 2, RT], f16, tag="d", name="d_last")
        nc.vector.tensor_sub(out=d_t[:], in0=hb_t[t][:], in1=n_t[t][:])
        e_t = wrk.tile([128, 2, RT], f16, tag="e", name="e_last")
        nc.vector.tensor_mul(out=e_t[:], in0=z_t[t][:], in1=d_t[:])
        ho_t[t] = outp.tile([128, 2, RT], f16, tag="ho", name="ho_last")
        nc.vector.tensor_add(out=ho_t[t][:], in0=n_t[t][:], in1=e_t[:])
        # out DMAs last on SP so their sem-waits never block input DMA issue
        for tt in range(NRT):
            nc.sync.dma_start(out=out_d[tt], in_=ho_t[tt][:])

    nc.compile()
    return nc


def _get_nc():
    if "nc" not in _compiled:
        _compiled["nc"] = _build_nc()
    return _compiled["nc"]


def _make_in_maps(h, X_obs, i_obs, W_ih, W_hh, b_ih, b_hh):
    import ml_dtypes

    f32 = np.float32
    f16 = np.float16
    f8 = ml_dtypes.float8_e4m3

    x = np.asarray(X_obs, f32).reshape(M, IN2)
    hs = np.asarray(h, f32)[np.asarray(i_obs)]
    W_ih = np.asarray(W_ih, f32)
    W_hh = np.asarray(W_hh, f32)
    b_ih = np.asarray(b_ih, f32)
    b_hh = np.asarray(b_hh, f32)

    wiT = W_ih.T * S          # [128, 768]
    whT = W_hh.T * S          # [256, 768]
    brz = (b_ih[: 2 * H] + b_hh[: 2 * H]) * S    # [512]
    bin_ = b_ih[2 * H :] * S                     # [256]
    bhn = b_hh[2 * H :] * S                      # [256]

    # wrz: per gate-half gh (r0,r1,z0,z1): groups [Wih-block, bias-block,
    # Whh-lo-block, Whh-hi-block], each [128 contract, 128 gates].
    wrz = np.zeros((128, 16, 128), f32)
    for gh in range(4):
        gs = slice(gh * 128, (gh + 1) * 128)
        wrz[:, 4 * gh + 0, :] = wiT[:, gs]
        wrz[0, 4 * gh + 1, :] = brz[gs]
        wrz[:, 4 * gh + 2, :] = whT[0:128, gs]
        wrz[:, 4 * gh + 3, :] = whT[128:256, gs]
    # win: per half j: [Wih_n-block, bias-block]
    win = np.zeros((128, 4, 128), f32)
    for j in range(2):
        gs = slice(2 * H + j * 128, 2 * H + (j + 1) * 128)
        win[:, 2 * j + 0, :] = wiT[:, gs]
        win[0, 2 * j + 1, :] = bin_[j * 128 : (j + 1) * 128]
    # whn: per half j: [Whh_n lo-block, hi-block]  (fp16)
    whn = np.zeros((128, 4, 128), f32)
    for j in range(2):
        gs = slice(2 * H + j * 128, 2 * H + (j + 1) * 128)
        whn[:, 2 * j + 0, :] = whT[0:128, gs]
        whn[:, 2 * j + 1, :] = whT[128:256, gs]

    wrz = wrz.astype(f8)
    win = win.astype(f8)
    whn = whn.astype(f16)
    ident = np.eye(128, dtype=f16)
    bhn2 = np.ascontiguousarray(bhn.reshape(2, 128).T)  # [128, 2]

    xT = x.T                   # [128, M]
    hT = hs.T                  # [256, M]
    in_maps = []
    for c in range(NCORES):
        cols = slice(c * MC, (c + 1) * MC)
        xc = xT[:, cols]       # [128, MC]
        hc = hT[:, cols]       # [256, MC]
        # u: [NRT, 128, 4, RT] groups (x, const, h_lo, h_hi) in fp8
        u = np.zeros((NRT, 128, 4, RT), f32)
        hb = np.empty((NRT, 128, 2, RT), f32)
        for t in range(NRT):
            cs = slice(t * RT, (t + 1) * RT)
            u[t, :, 0, :] = xc[:, cs]
            u[t, 0, 1, :] = 1.0
            u[t, :, 2, :] = hc[0:128, cs]
            u[t, :, 3, :] = hc[128:256, cs]
            hb[t, :, 0, :] = hc[0:128, cs]
            hb[t, :, 1, :] = hc[128:256, cs]
        in_maps.append(
            {
                "u": u.astype(f8),
                "hb": hb.astype(f16),
                "wrz": wrz,
                "win": win,
                "whn": whn,
                "ident": ident,
                "bhn": bhn2,
            }
        )
    return in_maps


def run_on_device(h, X_obs, i_obs, W_ih, W_hh, b_ih, b_hh, **run_kwargs):
    """Returns (h_new [M,H] fp32, BassKernelResults)."""
    from concourse.bass_utils import run_bass_kernel_spmd

    in_maps = _make_in_maps(h, X_obs, i_obs, W_ih, W_hh, b_ih, b_hh)
    res = run_bass_kernel_spmd(_get_nc(), in_maps, list(range(NCORES)), **run_kwargs)
    parts = []
    for r in res.results:
        o = np.asarray(r["hout"], np.float32)   # [NRT, 128, 2, RT]
        # [t, p, j, c] -> rows t*RT+c, dims j*128+p
        o = o.transpose(0, 3, 2, 1).reshape(MC, H)
        parts.append(o)
    h_new = np.concatenate(parts, axis=0)
    return h_new, res


def kernel(h, X_obs, i_obs, W_ih, W_hh, b_ih, b_hh):
    h = np.asarray(h, np.float32)
    i_obs = np.asarray(i_obs)
    h_new, _ = run_on_device(h, X_obs, i_obs, W_ih, W_hh, b_ih, b_hh)
    out = h.copy()
    out[i_obs] = h_new
    return out


# revision 7
# speedup vs baseline: 1.3408x; 1.1255x over previous
"""GRU observation-cell kernel for Trainium2 (8 NeuronCores).

Reference computation:
    x = X_obs.reshape(M, 128); hs = h[i_obs]
    h_new = GRUCell(x, hs)  (torch gate order r,z,n)
    out = h.at[i_obs].set(h_new)

Device strategy (data parallel over observed rows, per sharding hint):
  - M=20000 observed rows sharded 2500/core across 8 cores, tiled in 5
    column-tiles of 500 rows (gates-on-partitions layout).
  - r/z gates: fp8e4 DoubleRow matmuls (2x PE rate). Moving operand is
    u = [x^T; const; hs^T(lo); hs^T(hi)] (4 contract groups of 128); the
    const group carries a single 1.0 row so the r/z (and i_n) biases ride
    in the matmul as an extra contract row -> PSUM already contains
    16*(gates+bias), letting one activation op drain two PSUM banks
    (the per-partition activation bias could not express per-bank biases).
  - n gate: i_n via one fp8 DoubleRow matmul (bias folded the same way);
    h_n in fp16 (tanh has slope 1, keep it accurate); t1 = r*(h_n+b_hn)
    as scalar_tensor_tensor (j=0 on DVE, j=1 on GPSIMD); the i_n + t1 add
    rides the PE as an fp16 identity-matmul accumulate; tanh drains it.
  - All fp8/fp16 weights are pre-scaled by 16 on host (fp8e4 subnormal
    avoidance); activations apply scale=1/16.
  - Blend (1-z)n + z*hs: three fp16 tensor-tensor ops on DVE (2x mode).
  - Input DMAs issued from SP (HWDGE), output DMAs from GPSIMD (SWDGE)
    to spread descriptor-generation across both paths.
"""

import numpy as np

N, H, IN2, M, NCORES = 100000, 256, 128, 20000, 8
MC = M // NCORES        # 2500 observed rows per core
RT = 500                # rows per column-tile (<=512 fp32 PSUM bank limit)
NRT = MC // RT          # 5 row tiles per core
S = 16.0                # fp8 weight pre-scale; activations apply 1/S

_compiled = {}


def _build_nc():
    from contextlib import ExitStack

    from concourse import bacc
    import concourse.mybir as mybir
    from concourse.tile import TileContext

    dt = mybir.dt
    f32 = dt.float32
    f16 = dt.float16
    f8 = dt.float8e4
    AF = mybir.ActivationFunctionType
    ALU = mybir.AluOpType
    DR = mybir.MatmulPerfMode.DoubleRow

    nc = bacc.Bacc(None, target_bir_lowering=False)

    u_d = nc.dram_tensor("u", [NRT, 128, 4, RT], f8, kind="ExternalInput")
    hb_d = nc.dram_tensor("hb", [NRT, 128, 2, RT], f16, kind="ExternalInput")
    wrz_d = nc.dram_tensor("wrz", [128, 16, 128], f8, kind="ExternalInput")
    win_d = nc.dram_tensor("win", [128, 4, 128], f8, kind="ExternalInput")
    whn_d = nc.dram_tensor("whn", [128, 4, 128], f16, kind="ExternalInput")
    id_d = nc.dram_tensor("ident", [128, 128], f16, kind="ExternalInput")
    bhn_d = nc.dram_tensor("bhn", [128, 2], f32, kind="ExternalInput")
    out_d = nc.dram_tensor("hout", [NRT, 128, 2, RT], f16, kind="ExternalOutput")

    with TileContext(nc) as tc, ExitStack() as ctx:
        const = ctx.enter_context(tc.tile_pool(name="const", bufs=1))
        uin = ctx.enter_context(tc.tile_pool(name="uin", bufs=NRT))
        hin = ctx.enter_context(tc.tile_pool(name="hin", bufs=NRT))
        rzp = ctx.enter_context(tc.tile_pool(name="rzp", bufs=2))
        wrk = ctx.enter_context(tc.tile_pool(name="wrk", bufs=2))
        outp = ctx.enter_context(tc.tile_pool(name="outp", bufs=NRT))
        psum = ctx.enter_context(tc.tile_pool(name="psum", bufs=1, space="PSUM"))

        # --- all input DMAs up front (no WAR waits: bufs=NRT), critical first ---
        u_t = [None] * NRT
        hb_t = [None] * NRT
        u_t[0] = uin.tile([128, 4, RT], f8, tag="u", name="u0")
        nc.sync.dma_start(out=u_t[0][:], in_=u_d[0])
        wrz_sb = const.tile([128, 16, 128], f8, tag="wrz")
        nc.sync.dma_start(out=wrz_sb[:], in_=wrz_d[:, :, :])
        hb_t[0] = hin.tile([128, 2, RT], f16, tag="hb", name="hb0")
        nc.sync.dma_start(out=hb_t[0][:], in_=hb_d[0])
        win_sb = const.tile([128, 4, 128], f8, tag="win")
        nc.sync.dma_start(out=win_sb[:], in_=win_d[:, :, :])
        whn_sb = const.tile([128, 4, 128], f16, tag="whn")
        nc.sync.dma_start(out=whn_sb[:], in_=whn_d[:, :, :])
        bhn_sb = const.tile([128, 2], f32, tag="bhn")
        nc.sync.dma_start(out=bhn_sb[:], in_=bhn_d[:, :])
        id_sb = const.tile([128, 128], f16, tag="ident")
        nc.sync.dma_start(out=id_sb[:], in_=id_d[:, :])
        for tt in range(1, NRT):
            u_t[tt] = uin.tile([128, 4, RT], f8, tag="u", name=f"u{tt}")
            nc.sync.dma_start(out=u_t[tt][:], in_=u_d[tt])
            hb_t[tt] = hin.tile([128, 2, RT], f16, tag="hb", name=f"hb{tt}")
            nc.sync.dma_start(out=hb_t[tt][:], in_=hb_d[tt])

        r_t = [None] * NRT
        z_t = [None] * NRT
        n_t = [None] * NRT
        t1_t = [[None, None] for _ in range(NRT)]
        inps_t = [None] * NRT
        ho_t = [None] * NRT

        def rz_mms(t, gh, ps):
            # gh in 0..3 -> r0,r1,z0,z1; two DoubleRow matmuls: (x,const)+(h0,h1)
            nc.tensor.matmul(
                ps[:], lhsT=wrz_sb[:, 4 * gh : 4 * gh + 2, :],
                rhs=u_t[t][:, 0:2, :], start=True, stop=False, perf_mode=DR,
            )
            nc.tensor.matmul(
                ps[:], lhsT=wrz_sb[:, 4 * gh + 2 : 4 * gh + 4, :],
                rhs=u_t[t][:, 2:4, :], start=False, stop=True, perf_mode=DR,
            )

        def id_mms(t):
            # i_n PSUM accumulation finishes with the t1 identity-matmul ride
            for j in range(2):
                nc.tensor.matmul(
                    inps_t[t][:, j, 0:RT], lhsT=id_sb[:], rhs=t1_t[t][j][:],
                    start=False, stop=True,
                )

        def emit_in_id(tp):
            # i_n DoubleRow matmuls for tile tp, then the t1 identity ride.
            # Emitted one section late so the wait on B_{tp-1}'s PSUM free
            # never blocks the next tile's r/z matmuls in the PE stream.
            inps_t[tp] = psum.tile([128, 2, 512], f32, tag="inps", name="inps")
            for j in range(2):
                nc.tensor.matmul(
                    inps_t[tp][:, j, 0:RT], lhsT=win_sb[:, 2 * j : 2 * j + 2, :],
                    rhs=u_t[tp][:, 0:2, :], start=True, stop=False, perf_mode=DR,
                )
            id_mms(tp)

        def emit_b(tp):
            n_t[tp] = wrk.tile([128, 2, RT], f16, tag="n", name="n_t")
            nc.scalar.activation(
                out=n_t[tp][:], in_=inps_t[tp][:, :, 0:RT],
                func=AF.Tanh, scale=1.0 / S,
            )

        def emit_blend(tp, last):
            d_t = wrk.tile([128, 2, RT], f16, tag="d", name="d_t")
            nc.vector.tensor_sub(out=d_t[:], in0=hb_t[tp][:], in1=n_t[tp][:])
            e_t = wrk.tile([128, 2, RT], f16, tag="e", name="e_t")
            nc.vector.tensor_mul(out=e_t[:], in0=z_t[tp][:], in1=d_t[:])
            ho_t[tp] = outp.tile([128, 2, RT], f16, tag="ho", name="ho")
            if last:
                nc.vector.tensor_add(out=ho_t[tp][:], in0=n_t[tp][:], in1=e_t[:])
            else:
                # final add on the otherwise-idle GPSIMD engine (SBUF only)
                nc.gpsimd.tensor_add(out=ho_t[tp][:], in0=n_t[tp][:], in1=e_t[:])

        for t in range(NRT):
            # --- PE: r/z first (feed Act asap), then prior tile's n-path ---
            rps = psum.tile([128, 2, 512], f32, tag="rps", name="rps")
            zps = psum.tile([128, 2, 512], f32, tag="zps", name="zps")
            for j in range(2):
                rz_mms(t, j, rps[:, j, 0:RT])
            for j in range(2):
                rz_mms(t, 2 + j, zps[:, j, 0:RT])
            if t > 0:
                emit_in_id(t - 1)
            hnps = psum.tile([128, 2, 512], f32, tag="hnps", name="hnps")
            for j in range(2):
                for k in range(2):
                    nc.tensor.matmul(
                        hnps[:, j, 0:RT], lhsT=whn_sb[:, 2 * j + k, :],
                        rhs=hb_t[t][:, k, :], start=(k == 0), stop=(k == 1),
                    )

            # --- Act: R_t, Z_t, then B_{t-1} ---
            r_t[t] = rzp.tile([128, 2, RT], f16, tag="r", name="r_t")
            nc.scalar.activation(
                out=r_t[t][:], in_=rps[:, :, 0:RT], func=AF.Sigmoid, scale=1.0 / S
            )
            z_t[t] = rzp.tile([128, 2, RT], f16, tag="z", name="z_t")
            nc.scalar.activation(
                out=z_t[t][:], in_=zps[:, :, 0:RT], func=AF.Sigmoid, scale=1.0 / S
            )
            if t > 0:
                emit_b(t - 1)

            # --- DVE: t1_j = (h_n + 16*b_hn) * r (GPSIMD cannot read PSUM) ---
            for j in range(2):
                t1_t[t][j] = wrk.tile([128, RT], f16, tag=f"t1_{j}", name="t1")
                nc.vector.scalar_tensor_tensor(
                    out=t1_t[t][j][:], in0=hnps[:, j, 0:RT],
                    scalar=bhn_sb[:, j : j + 1], in1=r_t[t][:, j, :],
                    op0=ALU.add, op1=ALU.mult,
                )

            # --- blend of previous tile: ho = n + z*(hb - n) ---
            if t > 0:
                emit_blend(t - 1, last=False)

        # --- tail: last tile's n-path/blend, then all output DMAs from SP ---
        t = NRT - 1
        emit_in_id(t)
        emit_b(t)
        emit_blend(t, last=True)
        # out DMAs last on SP so their sem-waits never block input DMA issue
        for tt in range(NRT):
            nc.sync.dma_start(out=out_d[tt], in_=ho_t[tt][:])

    nc.compile()
    return nc


def _get_nc():
    if "nc" not in _compiled:
        _compiled["nc"] = _build_nc()
    return _compiled["nc"]


def _make_in_maps(h, X_obs, i_obs, W_ih, W_hh, b_ih, b_hh):
    import ml_dtypes

    f32 = np.float32
    f16 = np.float16
    f8 = ml_dtypes.float8_e4m3

    x = np.asarray(X_obs, f32).reshape(M, IN2)
    hs = np.asarray(h, f32)[np.asarray(i_obs)]
    W_ih = np.asarray(W_ih, f32)
    W_hh = np.asarray(W_hh, f32)
    b_ih = np.asarray(b_ih, f32)
    b_hh = np.asarray(b_hh, f32)

    wiT = W_ih.T * S          # [128, 768]
    whT = W_hh.T * S          # [256, 768]
    brz = (b_ih[: 2 * H] + b_hh[: 2 * H]) * S    # [512]
    bin_ = b_ih[2 * H :] * S                     # [256]
    bhn = b_hh[2 * H :] * S                      # [256]

    # wrz: per gate-half gh (r0,r1,z0,z1): groups [Wih-block, bias-block,
    # Whh-lo-block, Whh-hi-block], each [128 contract, 128 gates].
    wrz = np.zeros((128, 16, 128), f32)
    for gh in range(4):
        gs = slice(gh * 128, (gh + 1) * 128)
        wrz[:, 4 * gh + 0, :] = wiT[:, gs]
        wrz[0, 4 * gh + 1, :] = brz[gs]
        wrz[:, 4 * gh + 2, :] = whT[0:128, gs]
        wrz[:, 4 * gh + 3, :] = whT[128:256, gs]
    # win: per half j: [Wih_n-block, bias-block]
    win = np.zeros((128, 4, 128), f32)
    for j in range(2):
        gs = slice(2 * H + j * 128, 2 * H + (j + 1) * 128)
        win[:, 2 * j + 0, :] = wiT[:, gs]
        win[0, 2 * j + 1, :] = bin_[j * 128 : (j + 1) * 128]
    # whn: per half j: [Whh_n lo-block, hi-block]  (fp16)
    whn = np.zeros((128, 4, 128), f32)
    for j in range(2):
        gs = slice(2 * H + j * 128, 2 * H + (j + 1) * 128)
        whn[:, 2 * j + 0, :] = whT[0:128, gs]
        whn[:, 2 * j + 1, :] = whT[128:256, gs]

    wrz = wrz.astype(f8)
    win = win.astype(f8)
    whn = whn.astype(f16)
    ident = np.eye(128, dtype=f16)
    bhn2 = np.ascontiguousarray(bhn.reshape(2, 128).T)  # [128, 2]

    xT = x.T                   # [128, M]
    hT = hs.T                  # [256, M]
    in_maps = []
    for c in range(NCORES):
        cols = slice(c * MC, (c + 1) * MC)
        xc = xT[:, cols]       # [128, MC]
        hc = hT[:, cols]       # [256, MC]
        # u: [NRT, 128, 4, RT] groups (x, const, h_lo, h_hi) in fp8
        u = np.zeros((NRT, 128, 4, RT), f32)
        hb = np.empty((NRT, 128, 2, RT), f32)
        for t in range(NRT):
            cs = slice(t * RT, (t + 1) * RT)
            u[t, :, 0, :] = xc[:, cs]
            u[t, 0, 1, :] = 1.0
            u[t, :, 2, :] = hc[0:128, cs]
            u[t, :, 3, :] = hc[128:256, cs]
            hb[t, :, 0, :] = hc[0:128, cs]
            hb[t, :, 1, :] = hc[128:256, cs]
        in_maps.append(
            {
                "u": u.astype(f8),
                "hb": hb.astype(f16),
                "wrz": wrz,
                "win": win,
                "whn": whn,
                "ident": ident,
                "bhn": bhn2,
            }
        )
    return in_maps


def run_on_device(h, X_obs, i_obs, W_ih, W_hh, b_ih, b_hh, **run_kwargs):
    """Returns (h_new [M,H] fp32, BassKernelResults)."""
    from concourse.bass_utils import run_bass_kernel_spmd

    in_maps = _make_in_maps(h, X_obs, i_obs, W_ih, W_hh, b_ih, b_hh)
    res = run_bass_kernel_spmd(_get_nc(), in_maps, list(range(NCORES)), **run_kwargs)
    parts = []
    for r in res.results:
        o = np.asarray(r["hout"], np.float32)   # [NRT, 128, 2, RT]
        # [t, p, j, c] -> rows t*RT+c, dims j*128+p
        o = o.transpose(0, 3, 2, 1).reshape(MC, H)
        parts.append(o)
    h_new = np.concatenate(parts, axis=0)
    return h_new, res


def kernel(h, X_obs, i_obs, W_ih, W_hh, b_ih, b_hh):
    h = np.asarray(h, np.float32)
    i_obs = np.asarray(i_obs)
    h_new, _ = run_on_device(h, X_obs, i_obs, W_ih, W_hh, b_ih, b_hh)
    out = h.copy()
    out[i_obs] = h_new
    return out


# revision 13
# speedup vs baseline: 1.3946x; 1.0401x over previous
"""GRU observation-cell kernel for Trainium2 (8 NeuronCores).

Reference computation:
    x = X_obs.reshape(M, 128); hs = h[i_obs]
    h_new = GRUCell(x, hs)  (torch gate order r,z,n)
    out = h.at[i_obs].set(h_new)

Device strategy (data parallel over observed rows, per sharding hint):
  - M=20000 observed rows sharded 2500/core across 8 cores, tiled in 5
    column-tiles of 500 rows (gates-on-partitions layout).
  - r/z gates: fp8e4 DoubleRow matmuls (2x PE rate). Moving operand is
    u = [x^T; const; hs^T(lo); hs^T(hi)] (4 contract groups of 128); the
    const group carries a single 1.0 row so the r/z (and i_n) biases ride
    in the matmul as an extra contract row -> PSUM already contains
    16*(gates+bias), letting one activation op drain two PSUM banks
    (the per-partition activation bias could not express per-bank biases).
  - n gate: i_n via one fp8 DoubleRow matmul (bias folded the same way);
    h_n in fp16 (tanh has slope 1, keep it accurate); t1 = r*(h_n+b_hn)
    as scalar_tensor_tensor (j=0 on DVE, j=1 on GPSIMD); the i_n + t1 add
    rides the PE as an fp16 identity-matmul accumulate; tanh drains it.
  - All fp8/fp16 weights are pre-scaled by 16 on host (fp8e4 subnormal
    avoidance); activations apply scale=1/16.
  - Blend (1-z)n + z*hs: three fp16 tensor-tensor ops on DVE (2x mode).
  - Input DMAs issued from SP (HWDGE), output DMAs from GPSIMD (SWDGE)
    to spread descriptor-generation across both paths.
"""

import numpy as np

N, H, IN2, M, NCORES = 100000, 256, 128, 20000, 8
MC = M // NCORES        # 2500 observed rows per core
RT = 500                # rows per column-tile (<=512 fp32 PSUM bank limit)
NRT = MC // RT          # 5 row tiles per core
S = 16.0                # fp8 weight pre-scale; activations apply 1/S

_compiled = {}


def _build_nc():
    from contextlib import ExitStack

    from concourse import bacc
    import concourse.mybir as mybir
    from concourse.tile import TileContext

    dt = mybir.dt
    f32 = dt.float32
    f16 = dt.float16
    f8 = dt.float8e4
    AF = mybir.ActivationFunctionType
    ALU = mybir.AluOpType
    DR = mybir.MatmulPerfMode.DoubleRow

    nc = bacc.Bacc(None, target_bir_lowering=False)

    u_d = nc.dram_tensor("u", [NRT, 128, 4, RT], f8, kind="ExternalInput")
    hb_d = nc.dram_tensor("hb", [NRT, 128, 2, RT], f16, kind="ExternalInput")
    wrz_d = nc.dram_tensor("wrz", [128, 16, 128], f8, kind="ExternalInput")
    win_d = nc.dram_tensor("win", [128, 4, 128], f8, kind="ExternalInput")
    whn_d = nc.dram_tensor("whn", [128, 4, 128], f16, kind="ExternalInput")
    id_d = nc.dram_tensor("ident", [128, 128], f16, kind="ExternalInput")
    bhn_d = nc.dram_tensor("bhn", [128, 2], f32, kind="ExternalInput")
    out_d = nc.dram_tensor("hout", [NRT, 128, 2, RT], f16, kind="ExternalOutput")

    with TileContext(nc) as tc, ExitStack() as ctx:
        const = ctx.enter_context(tc.tile_pool(name="const", bufs=1))
        uin = ctx.enter_context(tc.tile_pool(name="uin", bufs=NRT))
        hin = ctx.enter_context(tc.tile_pool(name="hin", bufs=NRT))
        rzp = ctx.enter_context(tc.tile_pool(name="rzp", bufs=2))
        wrk = ctx.enter_context(tc.tile_pool(name="wrk", bufs=2))
        outp = ctx.enter_context(tc.tile_pool(name="outp", bufs=NRT))
        psum = ctx.enter_context(tc.tile_pool(name="psum", bufs=1, space="PSUM"))

        # --- all input DMAs up front (no WAR waits: bufs=NRT), critical first.
        # Split tile-0 input and the rz weights in halves so the first r/z
        # matmul pair can start after ~two small transfers.
        u_t = [None] * NRT
        hb_t = [None] * NRT
        u_t[0] = uin.tile([128, 4, RT], f8, tag="u", name="u0")
        wrz_sb = const.tile([128, 16, 128], f8, tag="wrz")
        nc.sync.dma_start(out=u_t[0][:, 0:2, :], in_=u_d[0, :, 0:2, :])
        nc.sync.dma_start(out=wrz_sb[:, 0:8, :], in_=wrz_d[:, 0:8, :])
        nc.sync.dma_start(out=u_t[0][:, 2:4, :], in_=u_d[0, :, 2:4, :])
        nc.sync.dma_start(out=wrz_sb[:, 8:16, :], in_=wrz_d[:, 8:16, :])
        u_t[1] = uin.tile([128, 4, RT], f8, tag="u", name="u1")
        nc.sync.dma_start(out=u_t[1][:], in_=u_d[1])
        hb_t[0] = hin.tile([128, 2, RT], f16, tag="hb", name="hb0")
        nc.sync.dma_start(out=hb_t[0][:], in_=hb_d[0])
        whn_sb = const.tile([128, 4, 128], f16, tag="whn")
        nc.sync.dma_start(out=whn_sb[:], in_=whn_d[:, :, :])
        bhn_sb = const.tile([128, 2], f32, tag="bhn")
        nc.sync.dma_start(out=bhn_sb[:], in_=bhn_d[:, :])
        win_sb = const.tile([128, 4, 128], f8, tag="win")
        nc.sync.dma_start(out=win_sb[:], in_=win_d[:, :, :])
        id_sb = const.tile([128, 128], f16, tag="ident")
        nc.sync.dma_start(out=id_sb[:], in_=id_d[:, :])
        for tt in range(1, NRT):
            if tt > 1:
                u_t[tt] = uin.tile([128, 4, RT], f8, tag="u", name=f"u{tt}")
                nc.sync.dma_start(out=u_t[tt][:], in_=u_d[tt])
            hb_t[tt] = hin.tile([128, 2, RT], f16, tag="hb", name=f"hb{tt}")
            nc.sync.dma_start(out=hb_t[tt][:], in_=hb_d[tt])

        r_t = [None] * NRT
        z_t = [None] * NRT
        n_t = [None] * NRT
        t1_t = [[None, None] for _ in range(NRT)]
        inps_t = [None] * NRT
        ho_t = [None] * NRT

        def rz_mms(t, gh, ps):
            # gh in 0..3 -> r0,r1,z0,z1; two DoubleRow matmuls: (x,const)+(h0,h1)
            # wrz layout: A-pairs (Wih,bias) at [2gh:2gh+2], B-pairs (Whh lo,hi)
            # at [8+2gh : 8+2gh+2] so the A half can be DMA'd first.
            nc.tensor.matmul(
                ps[:], lhsT=wrz_sb[:, 2 * gh : 2 * gh + 2, :],
                rhs=u_t[t][:, 0:2, :], start=True, stop=False, perf_mode=DR,
            )
            nc.tensor.matmul(
                ps[:], lhsT=wrz_sb[:, 8 + 2 * gh : 8 + 2 * gh + 2, :],
                rhs=u_t[t][:, 2:4, :], start=False, stop=True, perf_mode=DR,
            )

        def id_mms(t):
            # i_n PSUM accumulation finishes with the t1 identity-matmul ride
            for j in range(2):
                nc.tensor.matmul(
                    inps_t[t][:, j, 0:RT], lhsT=id_sb[:], rhs=t1_t[t][j][:],
                    start=False, stop=True,
                )

        def emit_in_id(tp, tag="inps"):
            # i_n DoubleRow matmuls for tile tp, then the t1 identity ride.
            # Emitted one section late so the wait on B_{tp-1}'s PSUM free
            # never blocks the next tile's r/z matmuls in the PE stream.
            inps_t[tp] = psum.tile([128, 2, 512], f32, tag=tag, name="inps")
            for j in range(2):
                nc.tensor.matmul(
                    inps_t[tp][:, j, 0:RT], lhsT=win_sb[:, 2 * j : 2 * j + 2, :],
                    rhs=u_t[tp][:, 0:2, :], start=True, stop=False, perf_mode=DR,
                )
            id_mms(tp)

        def emit_b(tp):
            n_t[tp] = wrk.tile([128, 2, RT], f16, tag="n", name="n_t")
            nc.scalar.activation(
                out=n_t[tp][:], in_=inps_t[tp][:, :, 0:RT],
                func=AF.Tanh, scale=1.0 / S,
            )

        def emit_blend(tp, on_dve):
            d_t = wrk.tile([128, 2, RT], f16, tag="d", name="d_t")
            nc.vector.tensor_sub(out=d_t[:], in0=hb_t[tp][:], in1=n_t[tp][:])
            e_t = wrk.tile([128, 2, RT], f16, tag="e", name="e_t")
            nc.vector.tensor_mul(out=e_t[:], in0=z_t[tp][:], in1=d_t[:])
            ho_t[tp] = outp.tile([128, 2, RT], f16, tag="ho", name="ho")
            if on_dve:
                nc.vector.tensor_add(out=ho_t[tp][:], in0=n_t[tp][:], in1=e_t[:])
            else:
                # final add on the otherwise-idle GPSIMD engine (SBUF only)
                nc.gpsimd.tensor_add(out=ho_t[tp][:], in0=n_t[tp][:], in1=e_t[:])

        for t in range(NRT):
            # --- PE: r/z first (feed Act asap), then prior tile's n-path ---
            rps = psum.tile([128, 2, 512], f32, tag="rps", name="rps")
            zps = psum.tile([128, 2, 512], f32, tag="zps", name="zps")
            for j in range(2):
                rz_mms(t, j, rps[:, j, 0:RT])
            for j in range(2):
                rz_mms(t, 2 + j, zps[:, j, 0:RT])
            if t > 0:
                emit_in_id(t - 1)
            hnps = psum.tile([128, 2, 512], f32, tag="hnps", name="hnps")
            for j in range(2):
                for k in range(2):
                    nc.tensor.matmul(
                        hnps[:, j, 0:RT], lhsT=whn_sb[:, 2 * j + k, :],
                        rhs=hb_t[t][:, k, :], start=(k == 0), stop=(k == 1),
                    )

            # --- Act: R_t, Z_t, then B_{t-1} ---
            r_t[t] = rzp.tile([128, 2, RT], f16, tag="r", name="r_t")
            nc.scalar.activation(
                out=r_t[t][:], in_=rps[:, :, 0:RT], func=AF.Sigmoid, scale=1.0 / S
            )
            z_t[t] = rzp.tile([128, 2, RT], f16, tag="z", name="z_t")
            nc.scalar.activation(
                out=z_t[t][:], in_=zps[:, :, 0:RT], func=AF.Sigmoid, scale=1.0 / S
            )
            if t > 0:
                emit_b(t - 1)

            # --- DVE: t1_j = (h_n + 16*b_hn) * r (GPSIMD cannot read PSUM) ---
            for j in range(2):
                t1_t[t][j] = wrk.tile([128, RT], f16, tag=f"t1_{j}", name="t1")
                nc.vector.scalar_tensor_tensor(
                    out=t1_t[t][j][:], in0=hnps[:, j, 0:RT],
                    scalar=bhn_sb[:, j : j + 1], in1=r_t[t][:, j, :],
                    op0=ALU.add, op1=ALU.mult,
                )

            # --- blend of previous tile: ho = n + z*(hb - n) ---
            if t > 0:
                # keep the late tiles' blend off the slow GPSIMD path so the
                # kernel tail is not gated on a 2.1us Pool op
                emit_blend(t - 1, on_dve=(t - 1 >= NRT - 2))

        # --- tail: last tile's n-path on the freed rz banks (avoids waiting
        # on B_{NRT-2}), then a j-half-interleaved blend + split output DMA ---
        t = NRT - 1
        emit_in_id(t, tag="rps")
        emit_b(t)
        ho_t[t] = outp.tile([128, 2, RT], f16, tag="ho", name="ho_last")
        for j in range(2):
            dh = wrk.tile([128, RT], f16, tag="dh", name="dh")
            nc.vector.tensor_sub(out=dh[:], in0=hb_t[t][:, j, :], in1=n_t[t][:, j, :])
            eh = wrk.tile([128, RT], f16, tag="eh", name="eh")
            nc.vector.tensor_mul(out=eh[:], in0=z_t[t][:, j, :], in1=dh[:])
            nc.vector.tensor_add(out=ho_t[t][:, j, :], in0=n_t[t][:, j, :], in1=eh[:])
        # out DMAs last on SP so their sem-waits never block input DMA issue
        for tt in range(NRT - 1):
            nc.sync.dma_start(out=out_d[tt], in_=ho_t[tt][:])
        for j in range(2):
            nc.sync.dma_start(out=out_d[t, :, j, :], in_=ho_t[t][:, j, :])

    nc.compile()
    return nc


def _get_nc():
    if "nc" not in _compiled:
        _compiled["nc"] = _build_nc()
    return _compiled["nc"]


def _make_in_maps(h, X_obs, i_obs, W_ih, W_hh, b_ih, b_hh):
    import ml_dtypes

    f32 = np.float32
    f16 = np.float16
    f8 = ml_dtypes.float8_e4m3

    x = np.asarray(X_obs, f32).reshape(M, IN2)
    hs = np.asarray(h, f32)[np.asarray(i_obs)]
    W_ih = np.asarray(W_ih, f32)
    W_hh = np.asarray(W_hh, f32)
    b_ih = np.asarray(b_ih, f32)
    b_hh = np.asarray(b_hh, f32)

    wiT = W_ih.T * S          # [128, 768]
    whT = W_hh.T * S          # [256, 768]
    brz = (b_ih[: 2 * H] + b_hh[: 2 * H]) * S    # [512]
    bin_ = b_ih[2 * H :] * S                     # [256]
    bhn = b_hh[2 * H :] * S                      # [256]

    # wrz: A-pairs [Wih-block, bias-block] for gh=r0,r1,z0,z1 in [0:8],
    # B-pairs [Whh-lo-block, Whh-hi-block] in [8:16].
    wrz = np.zeros((128, 16, 128), f32)
    for gh in range(4):
        gs = slice(gh * 128, (gh + 1) * 128)
        wrz[:, 2 * gh + 0, :] = wiT[:, gs]
        wrz[0, 2 * gh + 1, :] = brz[gs]
        wrz[:, 8 + 2 * gh + 0, :] = whT[0:128, gs]
        wrz[:, 8 + 2 * gh + 1, :] = whT[128:256, gs]
    # win: per half j: [Wih_n-block, bias-block]
    win = np.zeros((128, 4, 128), f32)
    for j in range(2):
        gs = slice(2 * H + j * 128, 2 * H + (j + 1) * 128)
        win[:, 2 * j + 0, :] = wiT[:, gs]
        win[0, 2 * j + 1, :] = bin_[j * 128 : (j + 1) * 128]
    # whn: per half j: [Whh_n lo-block, hi-block]  (fp16)
    whn = np.zeros((128, 4, 128), f32)
    for j in range(2):
        gs = slice(2 * H + j * 128, 2 * H + (j + 1) * 128)
        whn[:, 2 * j + 0, :] = whT[0:128, gs]
        whn[:, 2 * j + 1, :] = whT[128:256, gs]

    wrz = wrz.astype(f8)
    win = win.astype(f8)
    whn = whn.astype(f16)
    ident = np.eye(128, dtype=f16)
    bhn2 = np.ascontiguousarray(bhn.reshape(2, 128).T)  # [128, 2]

    xT = x.T                   # [128, M]
    hT = hs.T                  # [256, M]
    in_maps = []
    for c in range(NCORES):
        cols = slice(c * MC, (c + 1) * MC)
        xc = xT[:, cols]       # [128, MC]
        hc = hT[:, cols]       # [256, MC]
        # u: [NRT, 128, 4, RT] groups (x, const, h_lo, h_hi) in fp8
        u = np.zeros((NRT, 128, 4, RT), f32)
        hb = np.empty((NRT, 128, 2, RT), f32)
        for t in range(NRT):
            cs = slice(t * RT, (t + 1) * RT)
            u[t, :, 0, :] = xc[:, cs]
            u[t, 0, 1, :] = 1.0
            u[t, :, 2, :] = hc[0:128, cs]
            u[t, :, 3, :] = hc[128:256, cs]
            hb[t, :, 0, :] = hc[0:128, cs]
            hb[t, :, 1, :] = hc[128:256, cs]
        in_maps.append(
            {
                "u": u.astype(f8),
                "hb": hb.astype(f16),
                "wrz": wrz,
                "win": win,
                "whn": whn,
                "ident": ident,
                "bhn": bhn2,
            }
        )
    return in_maps


def run_on_device(h, X_obs, i_obs, W_ih, W_hh, b_ih, b_hh, **run_kwargs):
    """Returns (h_new [M,H] fp32, BassKernelResults)."""
    from concourse.bass_utils import run_bass_kernel_spmd

    in_maps = _make_in_maps(h, X_obs, i_obs, W_ih, W_hh, b_ih, b_hh)
    res = run_bass_kernel_spmd(_get_nc(), in_maps, list(range(NCORES)), **run_kwargs)
    parts = []
    for r in res.results:
        o = np.asarray(r["hout"], np.float32)   # [NRT, 128, 2, RT]
        # [t, p, j, c] -> rows t*RT+c, dims j*128+p
        o = o.transpose(0, 3, 2, 1).reshape(MC, H)
        parts.append(o)
    h_new = np.concatenate(parts, axis=0)
    return h_new, res


def kernel(h, X_obs, i_obs, W_ih, W_hh, b_ih, b_hh):
    h = np.asarray(h, np.float32)
    i_obs = np.asarray(i_obs)
    h_new, _ = run_on_device(h, X_obs, i_obs, W_ih, W_hh, b_ih, b_hh)
    out = h.copy()
    out[i_obs] = h_new
    return out
